# revision 19
# baseline (speedup 1.0000x reference)
"""Causal self-attention (B=2, T=4096, C=768, NH=12) on 8 trn2 cores.

Sharding: data-parallel over batch (2) x tensor-parallel over heads (12 -> 3
heads/core).  Core c handles batch c//4, heads 3*(c%4) .. 3*(c%4)+2.  Each
core computes qkv for its heads, causal attention, and its partial of the
output projection; a 4-core bf16 ReduceScatter per 512-row q-block reduces
the partials, each core keeps 128 rows per block, adds the proj bias, and
the host reassembles the stripes.

Key structure for PE density (the tensor engine only reaches 2.4 GHz after
~3us of gapless execution):
 - scores PSUM tiles are bf16 (1 bank each), freeing banks so the x-load/
   transpose/qkv-gen work for group g+1 is interleaved into the attention
   j-loops of earlier q-blocks as filler "thunks".
 - softmax normalize, projection, ReduceScatter and output staging for
   q-block qb are deferred into the attention loop of qb+1/qb+2 so the PE
   never waits on them.
 - the softmax reciprocal is broadcast across partitions on the otherwise
   idle GpSimd engine (partition_broadcast) instead of a broadcast DMA.

Engine budget: PE matmuls; ACT exp only; DVE casts/copies/masks/normalize;
GpSimd broadcasts + store-DMA issue + collectives; Sync load DMAs.
"""

import sys

if "/opt/trn_rl_repo" not in sys.path:
    sys.path.insert(0, "/opt/trn_rl_repo")

import numpy as np

B, T, C = 2, 4096, 768
NH, HD = 12, 64
N_CORES = 8
HPC = 3  # heads per core
TB = 512  # q block size
KT = 128  # kv tile size
NQB = T // TB  # 8 q blocks
NTT = T // KT  # 32 kv tiles
NCH = C // 128  # 6 contraction chunks
NCHUNK = 8  # reduce-scatter chunks (one per q block)
CH_ROWS = T // NCHUNK  # 512
SH_ROWS = CH_ROWS // 4  # 128 rows per core per chunk
SCALE = float(HD) ** -0.5

_CACHE = {}


def _build():
    if "nc" in _CACHE:
        return _CACHE["nc"]

    from concourse import bacc, tile, mybir

    dt = mybir.dt
    ActFn = mybir.ActivationFunctionType
    Alu = mybir.AluOpType

    nc = bacc.Bacc("TRN2", target_bir_lowering=False, debug=False,
                   num_devices=N_CORES)

    x_in = nc.dram_tensor("x", [T, C], dt.float32, kind="ExternalInput")
    wqk_in = nc.dram_tensor("wqk", [C, HPC * 128], dt.bfloat16, kind="ExternalInput")
    wv_in = nc.dram_tensor("wv", [C, HPC * 64], dt.bfloat16, kind="ExternalInput")
    wp_in = nc.dram_tensor("wp", [64, HPC * C], dt.bfloat16, kind="ExternalInput")
    bqk_in = nc.dram_tensor("bqk", [128, HPC], dt.float32, kind="ExternalInput")
    bv_in = nc.dram_tensor("bv", [1, HPC * 64], dt.bfloat16, kind="ExternalInput")
    bpr_in = nc.dram_tensor("bpr", [128, C], dt.float32, kind="ExternalInput")
    ident_in = nc.dram_tensor("ident", [128, 128], dt.float32, kind="ExternalInput")
    tmask_in = nc.dram_tensor("tmask", [128, 128], dt.bfloat16, kind="ExternalInput")
    out_ext = nc.dram_tensor("out", [NCHUNK * SH_ROWS, C], dt.float32,
                             kind="ExternalOutput")

    groups = [[0, 1, 2, 3], [4, 5, 6, 7]]

    with tile.TileContext(nc) as tc:
        with (
            tc.tile_pool(name="persist", bufs=1) as pp,
            tc.tile_pool(name="dram", bufs=1, space="DRAM") as dp,
            tc.tile_pool(name="stage", bufs=3) as sp,
            tc.tile_pool(name="ptp", bufs=4) as ptp,
            tc.tile_pool(name="ytp", bufs=3) as ytp,
            tc.tile_pool(name="bcp", bufs=3) as bcp,
            tc.tile_pool(name="stp", bufs=2) as stp,
            tc.tile_pool(name="osp", bufs=3) as osp,
            tc.tile_pool(name="shp", bufs=2) as shp,
            tc.tile_pool(name="psS", bufs=2, space="PSUM") as psS,
            tc.tile_pool(name="psB", bufs=2, space="PSUM") as psB,
            tc.tile_pool(name="psY", bufs=2, space="PSUM") as psY,
        ):
            # persistent SBUF tensors
            xT = pp.tile([128, NCH, T], dt.bfloat16, tag="xT")
            qT2 = pp.tile([128, T], dt.bfloat16, tag="qT2")
            kT2 = pp.tile([128, T], dt.bfloat16, tag="kT2")
            qT3 = pp.tile([64, T], dt.bfloat16, tag="qT3")
            kT3 = pp.tile([64, T], dt.bfloat16, tag="kT3")
            # vsb slot per (kv tile, head): [64 v cols | ones]
            vsb = pp.tile([128, NTT, HPC, 65], dt.bfloat16, tag="vsb")
            wqk = pp.tile([128, NCH, HPC * 128], dt.bfloat16, tag="wqk")
            wv = pp.tile([128, NCH, HPC * 64], dt.bfloat16, tag="wv")
            wp = pp.tile([64, HPC * C], dt.bfloat16, tag="wp")
            bqk = pp.tile([128, HPC], dt.float32, tag="bqk")
            bv = pp.tile([1, HPC * 64], dt.bfloat16, tag="bv")
            bpr = pp.tile([128, C], dt.float32, tag="bpr")
            ones = pp.tile([1, 128], dt.bfloat16, tag="ones")
            ident = pp.tile([128, 128], dt.float32, tag="ident")
            tmask = pp.tile([128, 128], dt.bfloat16, tag="tmask")

            cc_in = dp.tile([T, C], dt.bfloat16, tag="cc_in")
            cc_out = dp.tile([NCHUNK * SH_ROWS, C], dt.bfloat16, tag="cc_out")

            # ---- load weights / constants ----
            nc.sync.dma_start(ident[:], ident_in.ap()[:])
            nc.sync.dma_start(tmask[:], tmask_in.ap()[:])
            nc.sync.dma_start(bqk[:], bqk_in.ap()[:])
            nc.sync.dma_start(bv[:], bv_in.ap()[:])
            nc.sync.dma_start(bpr[:], bpr_in.ap()[:])
            nc.sync.dma_start(wp[:], wp_in.ap()[:])
            nc.vector.memset(ones[:], 1.0)
            nc.vector.memset(vsb[:], 1.0)  # ones columns; v cols overwritten
            for ci in range(NCH):
                nc.sync.dma_start(wqk[:, ci, :],
                                  wqk_in.ap()[ci * 128:(ci + 1) * 128, :])
                nc.sync.dma_start(wv[:, ci, :],
                                  wv_in.ap()[ci * 128:(ci + 1) * 128, :])

            yts = [None] * NQB  # yT tile per q block (for deferred proj)

            # ---- qkv-gen work items (thunks) ----
            def thunk_B(j):
                def run():
                    xf = sp.tile([128, C], dt.float32, tag="xf")
                    nc.sync.dma_start(xf[:], x_in.ap()[j * 128:(j + 1) * 128, :])
                    for ti, (c0, c1) in enumerate(((0, 4), (4, 6))):
                        tr = psB.tile([128, 512], dt.float32, tag="B")
                        for ci in range(c0, c1):
                            nc.tensor.transpose(
                                tr[:, (ci - c0) * 128:(ci - c0 + 1) * 128],
                                xf[:, ci * 128:(ci + 1) * 128], ident[:])
                        for ci in range(c0, c1):
                            nc.vector.tensor_copy(
                                xT[:, ci, j * 128:(j + 1) * 128],
                                tr[:, (ci - c0) * 128:(ci - c0 + 1) * 128])
                return run

            def thunk_C(j):
                def run():
                    pv = psB.tile([128, 512], dt.float32, tag="B")
                    for ci in range(NCH):
                        nc.tensor.matmul(
                            pv[:, 0:192], xT[:, ci, j * 128:(j + 1) * 128],
                            wv[:, ci, :], start=(ci == 0), stop=False)
                    nc.tensor.matmul(pv[:, 0:192], ones[:], bv[:],
                                     start=False, stop=True)
                    for hh in range(HPC):
                        nc.vector.tensor_copy(
                            vsb[:, j, hh, 0:64],
                            pv[:, hh * 64:(hh + 1) * 64])
                return run

            def thunk_D(hh, tb):
                def run():
                    tsl = slice(tb * TB, (tb + 1) * TB)
                    pq = psB.tile([128, 512], dt.float32, tag="B")
                    for ci in range(NCH):
                        nc.tensor.matmul(
                            pq[:],
                            wqk[:, ci, hh * 128:(hh + 1) * 128],
                            xT[:, ci, tsl],
                            start=(ci == 0), stop=(ci == NCH - 1))
                    st = stp.tile([128, TB], dt.bfloat16, tag="qkst")
                    nc.vector.tensor_scalar_add(st[:], pq[:], bqk[:, hh:hh + 1])
                    if hh < 2:
                        nc.sync.dma_start(qT2[hh * 64:(hh + 1) * 64, tsl], st[0:64, :])
                        nc.sync.dma_start(kT2[hh * 64:(hh + 1) * 64, tsl], st[64:128, :])
                    else:
                        nc.sync.dma_start(qT3[:, tsl], st[0:64, :])
                        nc.sync.dma_start(kT3[:, tsl], st[64:128, :])
                return run

            def group_thunks(g):
                th = []
                for j in range(8 * g, 8 * g + 4):
                    th.append(thunk_B(j))
                for hh in range(HPC):
                    th.append(thunk_D(hh, 2 * g))
                for j in range(8 * g, 8 * g + 4):
                    th.append(thunk_C(j))
                for j in range(8 * g + 4, 8 * g + 8):
                    th.append(thunk_B(j))
                for hh in range(HPC):
                    th.append(thunk_D(hh, 2 * g + 1))
                for j in range(8 * g + 4, 8 * g + 8):
                    th.append(thunk_C(j))
                return th

            fill = []  # pending thunks, drained inside attention loops

            def drain(k):
                for _ in range(min(k, len(fill))):
                    fill.pop(0)()

            # ---- deferred per-q-block epilogue ----
            def norm_head(yT, hh, py):
                rec = bcp.tile([65, TB], dt.float32, tag="rec")
                nc.vector.reciprocal(rec[64:65, :], py[64:65, :])
                sbb = bcp.tile([64, TB], dt.float32, tag="sbb")
                nc.gpsimd.dma_start(
                    sbb[:],
                    rec[64:65, :].unsqueeze(1).broadcast_to([1, 64, TB]))
                nc.vector.tensor_tensor(
                    yT[0:64, hh, :], py[0:64, :], sbb[:], op=Alu.mult)

            def emit_proj(qb):
                yT = yts[qb]
                for qs in range(4):
                    osb = osp.tile([128, C], dt.bfloat16, tag="osb")
                    for half in range(2):
                        pb = psB.tile([128, 512], dt.float32, tag="B")
                        for hh in range(HPC):
                            nc.tensor.matmul(
                                pb[:, 0:384],
                                yT[0:64, hh, qs * 128:(qs + 1) * 128],
                                wp[:, hh * C + half * 384: hh * C + half * 384 + 384],
                                start=(hh == 0), stop=(hh == HPC - 1))
                        nc.vector.tensor_copy(
                            osb[:, half * 384:half * 384 + 384], pb[:, 0:384])
                    nc.gpsimd.dma_start(
                        cc_in[qb * TB + qs * 128: qb * TB + (qs + 1) * 128, :],
                        osb[:])

            def emit_rs(qb):
                nc.gpsimd.collective_compute(
                    "ReduceScatter", Alu.add, replica_groups=groups,
                    ins=[cc_in[qb * CH_ROWS:(qb + 1) * CH_ROWS, :]],
                    outs=[cc_out[qb * SH_ROWS:(qb + 1) * SH_ROWS, :]])

            def emit_stage(qb):
                shi = shp.tile([128, C], dt.bfloat16, tag="shi")
                nc.sync.dma_start(shi[:], cc_out[qb * SH_ROWS:(qb + 1) * SH_ROWS, :])
                oso = osp.tile([128, C], dt.float32, tag="oso")
                nc.vector.tensor_tensor(oso[:], shi[:], bpr[:], op=Alu.add)
                nc.gpsimd.dma_start(
                    out_ext.ap()[qb * SH_ROWS:(qb + 1) * SH_ROWS, :], oso[:])

            # ---- attention per q block ----
            def attention(qb):
                n_kv = 4 * (qb + 1)
                diag0 = 4 * qb
                qsl = slice(qb * TB, (qb + 1) * TB)
                yT = ytp.tile([64, HPC, TB], dt.bfloat16, tag="yT")
                yts[qb] = yT

                # heads 0+1: row-packed
                py0 = psY.tile([65, TB], dt.float32, tag="py")
                py1 = psY.tile([65, TB], dt.float32, tag="py")
                for j in range(n_kv):
                    drain(1)
                    jsl = slice(j * KT, (j + 1) * KT)
                    ss = psS.tile([128, 1024], dt.float32, tag="S")
                    nc.tensor.matmul(ss[:, 0:TB], kT2[0:64, jsl],
                                     qT2[0:64, qsl], start=True, stop=True)
                    nc.tensor.matmul(ss[:, TB:2 * TB], kT2[64:128, jsl],
                                     qT2[64:128, qsl], start=True, stop=True)
                    pt = ptp.tile([128, 2 * TB], dt.bfloat16, tag="pt")
                    if j < diag0:
                        nc.scalar.activation(pt[:], ss[:], ActFn.Exp, scale=SCALE)
                    else:
                        k0 = (j - diag0) * KT
                        for u in range(2):
                            sl = slice(u * TB + k0, (u + 1) * TB)
                            nc.scalar.activation(pt[:, sl], ss[:, sl],
                                                 ActFn.Exp, scale=SCALE)
                            if k0 > 0:
                                nc.vector.memset(pt[:, u * TB: u * TB + k0], 0.0)
                            nc.vector.tensor_mul(
                                pt[:, u * TB + k0: u * TB + k0 + KT],
                                pt[:, u * TB + k0: u * TB + k0 + KT],
                                tmask[:])
                    nc.tensor.matmul(py0[:], vsb[:, j, 0, :], pt[:, 0:TB],
                                     start=(j == 0), stop=(j == n_kv - 1))
                    nc.tensor.matmul(py1[:], vsb[:, j, 1, :], pt[:, TB:2 * TB],
                                     start=(j == 0), stop=(j == n_kv - 1))
                norm_head(yT, 0, py0)
                norm_head(yT, 1, py1)

                # head 2: two kv tiles per pass
                py2 = psY.tile([65, TB], dt.float32, tag="py")
                for m in range(n_kv // 2):
                    drain(1)
                    ss = psS.tile([128, 1024], dt.float32, tag="S")
                    for u in range(2):
                        j = 2 * m + u
                        nc.tensor.matmul(
                            ss[:, u * TB:(u + 1) * TB],
                            kT3[:, j * KT:(j + 1) * KT],
                            qT3[:, qsl], start=True, stop=True)
                    pt = ptp.tile([128, 2 * TB], dt.bfloat16, tag="pt")
                    if 2 * m + 1 < diag0:
                        nc.scalar.activation(pt[:], ss[:], ActFn.Exp, scale=SCALE)
                    else:
                        for u in range(2):
                            j = 2 * m + u
                            k0 = (j - diag0) * KT
                            sl = slice(u * TB + k0, (u + 1) * TB)
                            nc.scalar.activation(pt[:, sl], ss[:, sl],
                                                 ActFn.Exp, scale=SCALE)
                            if k0 > 0:
                                nc.vector.memset(pt[:, u * TB: u * TB + k0], 0.0)
                            nc.vector.tensor_mul(
                                pt[:, u * TB + k0: u * TB + k0 + KT],
                                pt[:, u * TB + k0: u * TB + k0 + KT],
                                tmask[:])
                    for u in range(2):
                        j = 2 * m + u
                        nc.tensor.matmul(
                            py2[:], vsb[:, j, 2, :], pt[:, u * TB:(u + 1) * TB],
                            start=(j == 0), stop=(j == n_kv - 1))
                norm_head(yT, 2, py2)

                # deferred epilogues of earlier q blocks
                if qb >= 1:
                    emit_proj(qb - 1)
                    emit_rs(qb - 1)
                if qb >= 2:
                    emit_stage(qb - 2)

            # ---- schedule ----
            for th in group_thunks(0):
                th()
            fill.extend(group_thunks(1))
            attention(0)
            attention(1)
            drain(len(fill))
            fill.extend(group_thunks(2))
            attention(2)
            attention(3)
            drain(len(fill))
            fill.extend(group_thunks(3))
            attention(4)
            attention(5)
            drain(len(fill))
            attention(6)
            attention(7)
            emit_proj(7)
            emit_rs(7)
            emit_stage(6)
            emit_stage(7)

    nc.compile()
    _CACHE["nc"] = nc
    return nc


def _prep_core_inputs(x, w_attn, b_attn, w_proj, b_proj):
    """Host-side sharding: returns list of 8 input dicts."""
    import ml_dtypes

    bf16 = ml_dtypes.bfloat16
    ident = np.eye(128, dtype=np.float32)
    tmask = np.triu(np.ones((128, 128), np.float32)).astype(bf16)
    bpr = np.tile(b_proj.astype(np.float32)[None, :], (128, 1))
    in_maps = []
    for core in range(N_CORES):
        b = core // 4
        h0 = HPC * (core % 4)
        # wqk: per head [q cols | k cols] -> [768, 3*128]
        wqk = np.empty((C, HPC * 128), np.float32)
        bqk = np.empty((128, HPC), np.float32)
        wv = np.empty((C, HPC * 64), np.float32)
        bv = np.empty((1, HPC * 64), np.float32)
        wp = np.empty((64, HPC * C), np.float32)
        for hh in range(HPC):
            h = h0 + hh
            wqk[:, hh * 128: hh * 128 + 64] = w_attn[:, h * HD:(h + 1) * HD]
            wqk[:, hh * 128 + 64: hh * 128 + 128] = w_attn[:, C + h * HD: C + (h + 1) * HD]
            bqk[0:64, hh] = b_attn[h * HD:(h + 1) * HD]
            bqk[64:128, hh] = b_attn[C + h * HD: C + (h + 1) * HD]
            wv[:, hh * 64:(hh + 1) * 64] = w_attn[:, 2 * C + h * HD: 2 * C + (h + 1) * HD]
            bv[0, hh * 64:(hh + 1) * 64] = b_attn[2 * C + h * HD: 2 * C + (h + 1) * HD]
            wp[:, hh * C:(hh + 1) * C] = w_proj[h * HD:(h + 1) * HD, :]
        in_maps.append({
            "x": np.ascontiguousarray(x[b], np.float32),
            "wqk": wqk.astype(bf16),
            "wv": wv.astype(bf16),
            "wp": wp.astype(bf16),
            "bqk": bqk,
            "bv": bv.astype(bf16),
            "bpr": bpr,
            "ident": ident,
            "tmask": tmask,
        })
    return in_maps


def kernel(x, w_attn, b_attn, w_proj, b_proj, _trace=False, _trace_kwargs=None):
    x = np.asarray(x, np.float32)
    w_attn = np.asarray(w_attn, np.float32)
    b_attn = np.asarray(b_attn, np.float32)
    w_proj = np.asarray(w_proj, np.float32)
    b_proj = np.asarray(b_proj, np.float32)

    nc = _build()
    from concourse.bass_utils import run_bass_kernel_spmd

    in_maps = _prep_core_inputs(x, w_attn, b_attn, w_proj, b_proj)
    kw = dict(_trace_kwargs or {})
    res = run_bass_kernel_spmd(nc, in_maps, core_ids=list(range(N_CORES)),
                               trace=_trace, **kw)
    # reassemble: core 4*b + r holds, for each chunk c, global rows
    # c*512 + r*128 .. +128 in its out[c*128:(c+1)*128]
    out = np.empty((B, T, C), np.float32)
    for b in range(B):
        for r in range(4):
            o = res.results[4 * b + r]["out"]
            for ch in range(NCHUNK):
                g0 = ch * CH_ROWS + r * SH_ROWS
                out[b, g0:g0 + SH_ROWS] = o[ch * SH_ROWS:(ch + 1) * SH_ROWS]
    if _trace:
        kernel.last_results = res
    return out


# revision 39
# speedup vs baseline: 1.1591x; 1.1591x over previous
"""Causal self-attention (B=2, T=4096, C=768, NH=12) on 8 trn2 cores.

Sharding: data-parallel over batch (2) x tensor-parallel over heads (12 -> 3
heads/core).  Core c handles batch c//4, heads 3*(c%4) .. 3*(c%4)+2.  Each
core computes qkv for its heads, causal attention, and its partial of the
output projection; a 4-core bf16 ReduceScatter per 512-row q-block reduces
the partials, each core keeps 128 rows per block, adds the proj bias, and
the host reassembles the stripes.

Key structure for PE density (the tensor engine only reaches 2.4 GHz after
~3us of gapless execution):
 - scores PSUM tiles are bf16 (1 bank each), freeing banks so the x-load/
   transpose/qkv-gen work for group g+1 is interleaved into the attention
   j-loops of earlier q-blocks as filler "thunks".
 - softmax normalize, projection, ReduceScatter and output staging for
   q-block qb are deferred into the attention loop of qb+1/qb+2 so the PE
   never waits on them.
 - the softmax reciprocal is broadcast across partitions on the otherwise
   idle GpSimd engine (partition_broadcast) instead of a broadcast DMA.

Engine budget: PE matmuls; ACT exp only; DVE casts/copies/masks/normalize;
GpSimd broadcasts + store-DMA issue + collectives; Sync load DMAs.
"""

import sys

if "/opt/trn_rl_repo" not in sys.path:
    sys.path.insert(0, "/opt/trn_rl_repo")

import numpy as np

B, T, C = 2, 4096, 768
NH, HD = 12, 64
N_CORES = 8
HPC = 3  # heads per core
TB = 512  # q block size
KT = 128  # kv tile size
NQB = T // TB  # 8 q blocks
NTT = T // KT  # 32 kv tiles
NCH = C // 128  # 6 contraction chunks
NCHUNK = 8  # reduce-scatter chunks (one per q block)
CH_ROWS = T // NCHUNK  # 512
SH_ROWS = CH_ROWS // 4  # 128 rows per core per chunk
SCALE = float(HD) ** -0.5

_CACHE = {}


def _build():
    if "nc" in _CACHE:
        return _CACHE["nc"]

    from concourse import bacc, tile, mybir

    dt = mybir.dt
    ActFn = mybir.ActivationFunctionType
    Alu = mybir.AluOpType

    nc = bacc.Bacc("TRN2", target_bir_lowering=False, debug=False,
                   num_devices=N_CORES)

    x_in = nc.dram_tensor("x", [T, C], dt.float32, kind="ExternalInput")
    wqk_in = nc.dram_tensor("wqk", [C, HPC * 128], dt.bfloat16, kind="ExternalInput")
    wv_in = nc.dram_tensor("wv", [C, HPC * 64], dt.bfloat16, kind="ExternalInput")
    wp_in = nc.dram_tensor("wp", [64, HPC * C], dt.bfloat16, kind="ExternalInput")
    bqk_in = nc.dram_tensor("bqk", [128, HPC], dt.float32, kind="ExternalInput")
    bv_in = nc.dram_tensor("bv", [1, HPC * 64], dt.bfloat16, kind="ExternalInput")
    ident_in = nc.dram_tensor("ident", [128, 128], dt.float32, kind="ExternalInput")
    tmask_in = nc.dram_tensor("tmask", [128, 128], dt.bfloat16, kind="ExternalInput")
    out_ext = nc.dram_tensor("out", [NCHUNK * SH_ROWS, C], dt.bfloat16,
                             kind="ExternalOutput")

    groups = [[0, 1, 2, 3], [4, 5, 6, 7]]

    with tile.TileContext(nc) as tc:
        with (
            tc.tile_pool(name="persist", bufs=1) as pp,
            tc.tile_pool(name="dram", bufs=1, space="DRAM") as dp,
            tc.tile_pool(name="stage", bufs=3) as sp,
            tc.tile_pool(name="ptp", bufs=4) as ptp,
            tc.tile_pool(name="ytp", bufs=3) as ytp,
            tc.tile_pool(name="ynp", bufs=2) as ynp,
            tc.tile_pool(name="bcp", bufs=3) as bcp,
            tc.tile_pool(name="stp", bufs=2) as stp,
            tc.tile_pool(name="osp", bufs=3) as osp,
            tc.tile_pool(name="psS", bufs=2, space="PSUM") as psS,
            tc.tile_pool(name="psB", bufs=2, space="PSUM") as psB,
            tc.tile_pool(name="psY", bufs=2, space="PSUM") as psY,
        ):
            # persistent SBUF tensors
            xT = pp.tile([128, NCH, T], dt.bfloat16, tag="xT")
            qT2 = pp.tile([128, T], dt.bfloat16, tag="qT2")
            kT2 = pp.tile([128, T], dt.bfloat16, tag="kT2")
            qT3 = pp.tile([64, T], dt.bfloat16, tag="qT3")
            kT3 = pp.tile([64, T], dt.bfloat16, tag="kT3")
            # vsb slot per (kv tile, head): [64 v cols | ones]
            vsb = pp.tile([128, NTT, HPC, 65], dt.bfloat16, tag="vsb")
            wqk = pp.tile([128, NCH, HPC * 128], dt.bfloat16, tag="wqk")
            wv = pp.tile([128, NCH, HPC * 64], dt.bfloat16, tag="wv")
            wp = pp.tile([64, HPC * C], dt.bfloat16, tag="wp")
            bqk = pp.tile([128, HPC], dt.float32, tag="bqk")
            bv = pp.tile([1, HPC * 64], dt.bfloat16, tag="bv")
            ones = pp.tile([1, 128], dt.bfloat16, tag="ones")
            ident = pp.tile([128, 128], dt.float32, tag="ident")
            tmask = pp.tile([128, 128], dt.bfloat16, tag="tmask")

            cc_in = dp.tile([T, C], dt.bfloat16, tag="cc_in")
            cc_out = dp.tile([NCHUNK * SH_ROWS, C], dt.bfloat16, tag="cc_out")

            # ---- load constants/weights (ident first: transposes need it) ----
            nc.sync.dma_start(ident[:], ident_in.ap()[:])
            nc.sync.dma_start(tmask[:], tmask_in.ap()[:])
            nc.sync.dma_start(bqk[:], bqk_in.ap()[:])
            nc.sync.dma_start(bv[:], bv_in.ap()[:])
            nc.vector.memset(ones[:], 1.0)
            nc.vector.memset(vsb[:], 1.0)  # ones columns; v cols overwritten
            for ci in range(NCH):
                nc.sync.dma_start(wqk[:, ci, :],
                                  wqk_in.ap()[ci * 128:(ci + 1) * 128, :])
                nc.sync.dma_start(wv[:, ci, :],
                                  wv_in.ap()[ci * 128:(ci + 1) * 128, :])
            nc.sync.dma_start(wp[:], wp_in.ap()[:])

            yts = [None] * NQB  # yT tile per q block (for deferred proj)

            # ---- qkv-gen work items (thunks) ----
            def thunk_B(j):
                def run():
                    xf = sp.tile([128, C], dt.float32, tag="xf")
                    nc.sync.dma_start(xf[:], x_in.ap()[j * 128:(j + 1) * 128, :])
                    for ti, (c0, c1) in enumerate(((0, 4), (4, 6))):
                        tr = psB.tile([128, 512], dt.float32, tag="B")
                        for ci in range(c0, c1):
                            nc.tensor.transpose(
                                tr[:, (ci - c0) * 128:(ci - c0 + 1) * 128],
                                xf[:, ci * 128:(ci + 1) * 128], ident[:])
                        for ci in range(c0, c1):
                            nc.vector.tensor_copy(
                                xT[:, ci, j * 128:(j + 1) * 128],
                                tr[:, (ci - c0) * 128:(ci - c0 + 1) * 128])
                return run

            def thunk_C(j):
                def run():
                    pv = psB.tile([128, 512], dt.float32, tag="B")
                    for ci in range(NCH):
                        nc.tensor.matmul(
                            pv[:, 0:192], xT[:, ci, j * 128:(j + 1) * 128],
                            wv[:, ci, :], start=(ci == 0), stop=False)
                    nc.tensor.matmul(pv[:, 0:192], ones[:], bv[:],
                                     start=False, stop=True)
                    for hh in range(HPC):
                        nc.vector.tensor_copy(
                            vsb[:, j, hh, 0:64],
                            pv[:, hh * 64:(hh + 1) * 64])
                return run

            def thunk_D(hh, tb):
                def run():
                    tsl = slice(tb * TB, (tb + 1) * TB)
                    pq = psB.tile([128, 512], dt.float32, tag="B")
                    for ci in range(NCH):
                        nc.tensor.matmul(
                            pq[:],
                            wqk[:, ci, hh * 128:(hh + 1) * 128],
                            xT[:, ci, tsl],
                            start=(ci == 0), stop=(ci == NCH - 1))
                    st = stp.tile([128, TB], dt.bfloat16, tag="qkst")
                    nc.vector.tensor_scalar_add(st[:], pq[:], bqk[:, hh:hh + 1])
                    if hh < 2:
                        nc.sync.dma_start(qT2[hh * 64:(hh + 1) * 64, tsl], st[0:64, :])
                        nc.sync.dma_start(kT2[hh * 64:(hh + 1) * 64, tsl], st[64:128, :])
                    else:
                        nc.sync.dma_start(qT3[:, tsl], st[0:64, :])
                        nc.sync.dma_start(kT3[:, tsl], st[64:128, :])
                return run

            def group_thunks(g):
                th = []
                for j in range(8 * g, 8 * g + 4):
                    th.append(thunk_B(j))
                for hh in range(HPC):
                    th.append(thunk_D(hh, 2 * g))
                for j in range(8 * g, 8 * g + 4):
                    th.append(thunk_C(j))
                for j in range(8 * g + 4, 8 * g + 8):
                    th.append(thunk_B(j))
                for hh in range(HPC):
                    th.append(thunk_D(hh, 2 * g + 1))
                for j in range(8 * g + 4, 8 * g + 8):
                    th.append(thunk_C(j))
                return th

            fill = []  # pending thunks, drained inside attention loops

            def drain(k):
                for _ in range(min(k, len(fill))):
                    fill.pop(0)()

            # ---- deferred per-q-block epilogue ----
            def copy_head(yT, dn, hh, py):
                # fast copies that free the py PSUM slot; normalize happens
                # later (norm_head) off the PE-critical path
                nc.vector.tensor_copy(yT[0:64, hh, :], py[0:64, :])
                nc.vector.tensor_copy(dn[64:65, hh, :], py[64:65, :])

            def norm_head(yT, yN, dn, hh):
                rec = bcp.tile([65, TB], dt.float32, tag="rec")
                nc.vector.reciprocal(rec[64:65, :], dn[64:65, hh, :])
                sbb = bcp.tile([64, TB], dt.float32, tag="sbb")
                nc.gpsimd.dma_start(
                    sbb[:],
                    rec[64:65, :].unsqueeze(1).broadcast_to([1, 64, TB]))
                nc.vector.tensor_tensor(
                    yN[0:64, hh, :], yT[0:64, hh, :], sbb[:], op=Alu.mult)

            def emit_proj(qb):
                yN = yts[qb]
                for qs in range(4):
                    osb = osp.tile([128, C], dt.bfloat16, tag="osb")
                    for half in range(2):
                        pb = psB.tile([128, 512], dt.float32, tag="B")
                        for hh in range(HPC):
                            nc.tensor.matmul(
                                pb[:, 0:384],
                                yN[0:64, hh, qs * 128:(qs + 1) * 128],
                                wp[:, hh * C + half * 384: hh * C + half * 384 + 384],
                                start=(hh == 0), stop=(hh == HPC - 1))
                        nc.vector.tensor_copy(
                            osb[:, half * 384:half * 384 + 384], pb[:, 0:384])
                    nc.gpsimd.dma_start(
                        cc_in[qb * TB + qs * 128: qb * TB + (qs + 1) * 128, :],
                        osb[:])

            def emit_rs(qb):
                nc.gpsimd.collective_compute(
                    "ReduceScatter", Alu.add, replica_groups=groups,
                    ins=[cc_in[qb * CH_ROWS:(qb + 1) * CH_ROWS, :]],
                    outs=[cc_out[qb * SH_ROWS:(qb + 1) * SH_ROWS, :]])

            # ---- attention per q block ----
            def attention(qb):
                n_kv = 4 * (qb + 1)
                diag0 = 4 * qb
                qsl = slice(qb * TB, (qb + 1) * TB)
                yT = ytp.tile([64, HPC, TB], dt.bfloat16, tag="yT")
                dn = ytp.tile([65, HPC, TB], dt.float32, tag="dn")
                yN = ynp.tile([64, HPC, TB], dt.bfloat16, tag="yN")
                yts[qb] = yN

                # heads 0+1: row-packed; software-pipelined so PV(j) is
                # emitted after QK(j+1) and never stalls the PE on exp(j)
                py0 = psY.tile([65, TB], dt.float32, tag="py")
                py1 = psY.tile([65, TB], dt.float32, tag="py")

                def pv01(pj, ppt):
                    nc.tensor.matmul(py0[:], vsb[:, pj, 0, :], ppt[:, 0:TB],
                                     start=(pj == 0), stop=(pj == n_kv - 1))
                    nc.tensor.matmul(py1[:], vsb[:, pj, 1, :], ppt[:, TB:2 * TB],
                                     start=(pj == 0), stop=(pj == n_kv - 1))

                pend = None
                for j in range(n_kv):
                    drain(1)
                    jsl = slice(j * KT, (j + 1) * KT)
                    ss = psS.tile([128, 1024], dt.float32, tag="S")
                    nc.tensor.matmul(ss[:, 0:TB], kT2[0:64, jsl],
                                     qT2[0:64, qsl], start=True, stop=True)
                    nc.tensor.matmul(ss[:, TB:2 * TB], kT2[64:128, jsl],
                                     qT2[64:128, qsl], start=True, stop=True)
                    pt = ptp.tile([128, 2 * TB], dt.bfloat16, tag="pt")
                    if j < diag0:
                        nc.scalar.activation(pt[:], ss[:], ActFn.Exp, scale=SCALE)
                    else:
                        k0 = (j - diag0) * KT
                        for u in range(2):
                            sl = slice(u * TB + k0, (u + 1) * TB)
                            nc.scalar.activation(pt[:, sl], ss[:, sl],
                                                 ActFn.Exp, scale=SCALE)
                            if k0 > 0:
                                nc.vector.memset(pt[:, u * TB: u * TB + k0], 0.0)
                            nc.vector.tensor_mul(
                                pt[:, u * TB + k0: u * TB + k0 + KT],
                                pt[:, u * TB + k0: u * TB + k0 + KT],
                                tmask[:])
                    if pend is not None:
                        pv01(*pend)
                    pend = (j, pt)
                pv01(*pend)
                copy_head(yT, dn, 0, py0)
                copy_head(yT, dn, 1, py1)

                # head 2: two kv tiles per pass, same software pipeline
                py2 = psY.tile([65, TB], dt.float32, tag="py")

                def pv2(pm, ppt):
                    for u in range(2):
                        j = 2 * pm + u
                        nc.tensor.matmul(
                            py2[:], vsb[:, j, 2, :], ppt[:, u * TB:(u + 1) * TB],
                            start=(j == 0), stop=(j == n_kv - 1))

                pend = None
                for m in range(n_kv // 2):
                    drain(1)
                    ss = psS.tile([128, 1024], dt.float32, tag="S")
                    for u in range(2):
                        j = 2 * m + u
                        nc.tensor.matmul(
                            ss[:, u * TB:(u + 1) * TB],
                            kT3[:, j * KT:(j + 1) * KT],
                            qT3[:, qsl], start=True, stop=True)
                    pt = ptp.tile([128, 2 * TB], dt.bfloat16, tag="pt")
                    if 2 * m + 1 < diag0:
                        nc.scalar.activation(pt[:], ss[:], ActFn.Exp, scale=SCALE)
                    else:
                        for u in range(2):
                            j = 2 * m + u
                            k0 = (j - diag0) * KT
                            sl = slice(u * TB + k0, (u + 1) * TB)
                            nc.scalar.activation(pt[:, sl], ss[:, sl],
                                                 ActFn.Exp, scale=SCALE)
                            if k0 > 0:
                                nc.vector.memset(pt[:, u * TB: u * TB + k0], 0.0)
                            nc.vector.tensor_mul(
                                pt[:, u * TB + k0: u * TB + k0 + KT],
                                pt[:, u * TB + k0: u * TB + k0 + KT],
                                tmask[:])
                    if pend is not None:
                        pv2(*pend)
                    pend = (m, pt)
                pv2(*pend)
                copy_head(yT, dn, 2, py2)
                for hh in range(HPC):
                    norm_head(yT, yN, dn, hh)

                # deferred epilogues of earlier q blocks
                if qb >= 1:
                    emit_proj(qb - 1)
                    emit_rs(qb - 1)

            # ---- schedule ----
            for th in group_thunks(0):
                th()
            fill.extend(group_thunks(1))
            attention(0)
            attention(1)
            drain(len(fill))
            fill.extend(group_thunks(2))
            attention(2)
            attention(3)
            drain(len(fill))
            fill.extend(group_thunks(3))
            attention(4)
            attention(5)
            drain(len(fill))
            attention(6)
            attention(7)
            emit_proj(7)
            emit_rs(7)
            # final copies at the end: their RS-completion waits block nothing
            for qb in range(NQB):
                nc.sync.dma_start(
                    out_ext.ap()[qb * SH_ROWS:(qb + 1) * SH_ROWS, :],
                    cc_out[qb * SH_ROWS:(qb + 1) * SH_ROWS, :])

    nc.compile()
    _CACHE["nc"] = nc
    return nc


def _prep_core_inputs(x, w_attn, b_attn, w_proj, b_proj):
    """Host-side sharding: returns list of 8 input dicts."""
    import ml_dtypes

    bf16 = ml_dtypes.bfloat16
    ident = np.eye(128, dtype=np.float32)
    tmask = np.triu(np.ones((128, 128), np.float32)).astype(bf16)
    in_maps = []
    for core in range(N_CORES):
        b = core // 4
        h0 = HPC * (core % 4)
        # wqk: per head [q cols | k cols] -> [768, 3*128]
        wqk = np.empty((C, HPC * 128), np.float32)
        bqk = np.empty((128, HPC), np.float32)
        wv = np.empty((C, HPC * 64), np.float32)
        bv = np.empty((1, HPC * 64), np.float32)
        wp = np.empty((64, HPC * C), np.float32)
        for hh in range(HPC):
            h = h0 + hh
            wqk[:, hh * 128: hh * 128 + 64] = w_attn[:, h * HD:(h + 1) * HD]
            wqk[:, hh * 128 + 64: hh * 128 + 128] = w_attn[:, C + h * HD: C + (h + 1) * HD]
            bqk[0:64, hh] = b_attn[h * HD:(h + 1) * HD]
            bqk[64:128, hh] = b_attn[C + h * HD: C + (h + 1) * HD]
            wv[:, hh * 64:(hh + 1) * 64] = w_attn[:, 2 * C + h * HD: 2 * C + (h + 1) * HD]
            bv[0, hh * 64:(hh + 1) * 64] = b_attn[2 * C + h * HD: 2 * C + (h + 1) * HD]
            wp[:, hh * C:(hh + 1) * C] = w_proj[h * HD:(h + 1) * HD, :]
        in_maps.append({
            "x": np.ascontiguousarray(x[b], np.float32),
            "wqk": wqk.astype(bf16),
            "wv": wv.astype(bf16),
            "wp": wp.astype(bf16),
            "bqk": bqk,
            "bv": bv.astype(bf16),
            "ident": ident,
            "tmask": tmask,
        })
    return in_maps


def kernel(x, w_attn, b_attn, w_proj, b_proj, _trace=False, _trace_kwargs=None):
    x = np.asarray(x, np.float32)
    w_attn = np.asarray(w_attn, np.float32)
    b_attn = np.asarray(b_attn, np.float32)
    w_proj = np.asarray(w_proj, np.float32)
    b_proj = np.asarray(b_proj, np.float32)

    nc = _build()
    from concourse.bass_utils import run_bass_kernel_spmd

    in_maps = _prep_core_inputs(x, w_attn, b_attn, w_proj, b_proj)
    kw = dict(_trace_kwargs or {})
    res = run_bass_kernel_spmd(nc, in_maps, core_ids=list(range(N_CORES)),
                               trace=_trace, **kw)
    # reassemble: core 4*b + r holds, for each chunk c, global rows
    # c*512 + r*128 .. +128 in its out[c*128:(c+1)*128]
    out = np.empty((B, T, C), np.float32)
    bp = b_proj[None, :]
    for b in range(B):
        for r in range(4):
            o = np.asarray(res.results[4 * b + r]["out"], np.float32)
            for ch in range(NCHUNK):
                g0 = ch * CH_ROWS + r * SH_ROWS
                out[b, g0:g0 + SH_ROWS] = o[ch * SH_ROWS:(ch + 1) * SH_ROWS] + bp
    if _trace:
        kernel.last_results = res
    return out
